# revision 1
# baseline (speedup 1.0000x reference)
"""CapsuleLayer dynamic-routing kernel for 8 Trainium2 NeuronCores.

Sharding: data-parallel over batch (16 batches/core), weight replicated.
  u_hat[b,c,n,s] = sum_i W[c,n,s,i] * x[b,i,c]   (PE, fp32r, block-diag x)
  3 routing iterations; the b_ij update takes a mean over the full batch
  via one AllReduce per iteration (skipped on the last).

On-chip: u_hat kept in SBUF as A[c%128, chunk, b, n, s] (9 chunks of 128
channels).  s_j is a PE pass (c_ij stationary, u_hat moving); the
agreement <u_hat, v> is a GPSIMD multiply + DVE segmented reduce.
"""

import sys

sys.path.insert(0, "/opt/trn_rl_repo")

import numpy as np

B, IN_UNIT, IN_CHANNEL = 128, 16, 1152
NUM_UNIT, UNIT_SIZE = 16, 16
NCORES = 8
BL = B // NCORES               # 16 batches per core
NGROUP = IN_CHANNEL // 8       # 144 groups of 8 channels
NCHUNK = IN_CHANNEL // 128     # 9 c-chunks
NS = NUM_UNIT * UNIT_SIZE      # 256
FREE = BL * NS                 # 4096 = (b, n, s) free size per chunk

_cache = {}


def _build(single_core=False, niters=3, skip_prod=False):
    import concourse.bass as bass
    import concourse.bacc as bacc
    import concourse.mybir as mybir
    import concourse.tile as tile

    f32 = mybir.dt.float32
    f32r = mybir.dt.float32r
    ALU = mybir.AluOpType
    AX = mybir.AxisListType

    def sub(ap, off, dims, cast=None):
        a = bass.AP(ap.tensor, ap.offset + off, [list(d) for d in dims])
        return a.bitcast(cast) if cast is not None else a

    nc = bacc.Bacc("TRN2", target_bir_lowering=False, debug=False,
                   num_devices=1 if single_core else NCORES)

    wr_t = nc.dram_tensor("wr", [NGROUP * 128, 256], f32, kind="ExternalInput")
    xc_t = nc.dram_tensor("xc", [IN_CHANNEL, IN_UNIT, BL], f32, kind="ExternalInput")
    cij1_t = nc.dram_tensor("cij1", [128, NUM_UNIT], f32, kind="ExternalInput")
    vout_t = nc.dram_tensor("vout", [NUM_UNIT, BL * UNIT_SIZE], f32,
                            kind="ExternalOutput")

    with tile.TileContext(nc) as tc:
        with tc.tile_pool(name="apool", bufs=1) as apool, \
             tc.tile_pool(name="persist", bufs=1) as persist, \
             tc.tile_pool(name="drampool", bufs=1, space="DRAM") as drampool:
            # u_hat, fp32r, [c_part, chunk, b, n, s]
            A = apool.tile([128, NCHUNK, BL, NUM_UNIT, UNIT_SIZE], f32r)
            Aap = A[:]
            pstA = Aap.ap[0][0]
            b_ij = persist.tile([128, NCHUNK, NUM_UNIT], f32)
            cij_u = persist.tile([128, NUM_UNIT], f32r)   # uniform 1/16
            nc.gpsimd.memset(b_ij[:], 0.0)
            nc.sync.dma_start(cij_u[:], cij1_t[:].bitcast(f32r))

            # ---------------- production ----------------
            uhd = drampool.tile([NGROUP * 128, 256], f32)    # u_hat bounce
            with tc.tile_pool(name="bdp", bufs=1) as bdp, \
                 tc.tile_pool(name="wp", bufs=1) as wp, \
                 tc.tile_pool(name="stgp", bufs=1) as stgp, \
                 tc.tile_pool(name="psp", bufs=8, space="PSUM") as psp:
                for sg in (range(NCHUNK) if not skip_prod else []):
                    bd16 = bdp.tile([128, 16, 128], f32r, tag="bd16",
                                    name=f"bd16_{sg}")
                    pstB = bd16[:].ap[0][0]
                    if sg < 1:      # single slot; zero padding persists
                        nc.gpsimd.memset(bd16[:].bitcast(f32), 0.0)
                    for cc in range(8):
                        # block-diag xT: bd16[(cc,i), g, cc*16+b]
                        src = sub(xc_t[:], (8 * 16 * sg + cc) * IN_UNIT * BL,
                                  [[BL, IN_UNIT],
                                   [8 * IN_UNIT * BL, 16],
                                   [1, BL]], cast=f32r)
                        dst = sub(bd16[:], cc * 16 * pstB + cc * 16,
                                  [[pstB, IN_UNIT], [128, 16], [1, BL]])
                        nc.sync.dma_start(dst, src)
                    wts = []
                    for gq in range(4):
                        wt4 = wp.tile([128, 4, 256], f32r, tag=f"w{gq}",
                                      name=f"w{gq}_{sg}")
                        nc.sync.dma_start(
                            wt4[:], sub(wr_t[:], (sg * 16 + gq * 4) * 128 * 256,
                                        [[256, 128], [128 * 256, 4], [1, 256]],
                                        cast=f32r))
                        wts.append(wt4)
                    for half in range(2):
                        stgb = stgp.tile([128, 8, 256], f32, tag="stgb",
                                         bufs=2, name=f"stgb_{sg}_{half}")
                        for g8 in range(8):
                            gg = half * 8 + g8
                            ps = psp.tile([128, 256], f32, tag="ps",
                                          name=f"ps_{sg}_{gg}")
                            nc.tensor.matmul(ps[:], bd16[:, gg, :],
                                             wts[gg // 4][:, gg % 4, :],
                                             start=True, stop=True)
                            if gg % 2 == 0:
                                nc.vector.tensor_copy(stgb[:, g8, :], ps[:])
                            else:
                                nc.scalar.copy(stgb[:, g8, :], ps[:])
                        # (cc,b),(g,n,s) -> DRAM uhd[(g,cc,b), (n,s)]
                        dstu = sub(uhd[:], sg * 16 * 128 * 256
                                   + half * 8 * 8 * 16 * 256,
                                   [[16 * 256, 8], [256, 16],
                                    [8 * 16 * 256, 8], [1, 256]])
                        nc.sync.dma_start(dstu, stgb[:])
                    # readback c-partitioned: A[p, sg, (b,n,s)]
                    dstA = sub(Aap, sg * FREE, [[pstA, 128], [1, FREE]],
                               cast=f32)
                    srcu = sub(uhd[:], sg * 16 * 128 * 256,
                               [[FREE, 128], [1, FREE]])
                    nc.sync.dma_start(dstA, srcu)

            # ---------------- routing ----------------
            with tc.tile_pool(name="rt", bufs=1) as rt, \
                 tc.tile_pool(name="tb", bufs=2) as tb, \
                 tc.tile_pool(name="pss", bufs=1, space="PSUM") as pss:
                vb = rt.tile([128, FREE], f32)
                cij = rt.tile([128, NCHUNK, NUM_UNIT], f32)
                cij_r = rt.tile([128, NCHUNK, NUM_UNIT], f32r)
                smax = rt.tile([128, NCHUNK], f32)
                ssum = rt.tile([128, NCHUNK], f32)
                uv = rt.tile([128, NCHUNK, NUM_UNIT], f32)
                ar_sb = rt.tile([128, NCHUNK, NUM_UNIT], f32)

                for it in range(niters):
                    if it > 0:
                        # softmax over n of b_ij -> cij (f32r via DMA recast)
                        nc.vector.tensor_reduce(smax[:], b_ij[:], axis=AX.X,
                                                op=ALU.max)
                        mb = sub(smax[:], 0,
                                 [[NCHUNK, 128], [1, NCHUNK], [0, NUM_UNIT]])
                        nc.vector.tensor_tensor(cij[:], b_ij[:], mb,
                                                op=ALU.subtract)
                        nc.scalar.activation(cij[:], cij[:],
                                             mybir.ActivationFunctionType.Exp)
                        nc.vector.tensor_reduce(ssum[:], cij[:], axis=AX.X,
                                                op=ALU.add)
                        nc.vector.reciprocal(ssum[:], ssum[:])
                        sb = sub(ssum[:], 0,
                                 [[NCHUNK, 128], [1, NCHUNK], [0, NUM_UNIT]])
                        nc.vector.tensor_tensor(cij[:], cij[:], sb, op=ALU.mult)
                        nc.sync.dma_start(cij_r[:], cij[:].bitcast(f32r))

                    # s_j: PE pass, c_ij stationary, u_hat moving
                    psj = pss.tile([NUM_UNIT, FREE], f32, tag="psj",
                                   name=f"psj_{it}")
                    pstP = psj[:].ap[0][0]
                    for k in range(NCHUNK):
                        lhs = cij_u[:] if it == 0 else cij_r[:, k, :]
                        for j in range(FREE // 512):
                            nc.tensor.matmul(
                                psj[:, j * 512:(j + 1) * 512], lhs,
                                sub(Aap, k * FREE + j * 512,
                                    [[pstA, 128], [1, 512]]),
                                start=(k == 0), stop=(k == NCHUNK - 1))

                    # diagonal extract: s[n,(b,s)] = psj[n, (b,n,s)]
                    sjf = tb.tile([NUM_UNIT, FREE], f32, tag="big1", bufs=1,
                                  name=f"sjf_{it}")
                    nc.vector.tensor_copy(sjf[:, :FREE // 2],
                                          psj[:, :FREE // 2])
                    nc.scalar.copy(sjf[:, FREE // 2:], psj[:, FREE // 2:])
                    pstS = sjf[:].ap[0][0]
                    s_t = tb.tile([NUM_UNIT, BL, UNIT_SIZE], f32, tag="s_t",
                                  name=f"s_t{it}")
                    pstST = s_t[:].ap[0][0]
                    # one DMA: partition-dim step carries the diagonal offset
                    src = sub(sjf[:], 0,
                              [[pstS + UNIT_SIZE, NUM_UNIT],
                               [NS, BL], [1, UNIT_SIZE]])
                    nc.sync.dma_start(s_t[:], src)

                    # squash over s
                    s2 = tb.tile([NUM_UNIT, BL, UNIT_SIZE], f32, tag="big1",
                                 bufs=1, name=f"s2_{it}")
                    nc.vector.tensor_tensor(s2[:], s_t[:], s_t[:], op=ALU.mult)
                    sq = tb.tile([NUM_UNIT, BL], f32, tag="sq", name=f"sq_{it}")
                    nc.vector.tensor_reduce(sq[:], s2[:], axis=AX.X, op=ALU.add)
                    rsq = tb.tile([NUM_UNIT, BL], f32, tag="rsq",
                                  name=f"rsq_{it}")
                    nc.scalar.sqrt(rsq[:], sq[:])
                    den = tb.tile([NUM_UNIT, BL], f32, tag="den",
                                  name=f"den_{it}")
                    nc.vector.scalar_tensor_tensor(den[:], sq[:], 1.0, rsq[:],
                                                   op0=ALU.add, op1=ALU.mult)
                    nc.vector.reciprocal(den[:], den[:])
                    fac = tb.tile([NUM_UNIT, BL], f32, tag="fac",
                                  name=f"fac_{it}")
                    nc.vector.tensor_tensor(fac[:], sq[:], den[:], op=ALU.mult)
                    v_t = tb.tile([NUM_UNIT, BL, UNIT_SIZE], f32, tag="v_t",
                                  name=f"v_t{it}")
                    pstF = fac[:].ap[0][0]
                    fb = sub(fac[:], 0, [[pstF, NUM_UNIT], [1, BL],
                                         [0, UNIT_SIZE]])
                    nc.vector.tensor_tensor(v_t[:], s_t[:], fb, op=ALU.mult)

                    if it == niters - 1:
                        nc.sync.dma_start(vout_t[:],
                                          sub(v_t[:], 0,
                                              [[v_t[:].ap[0][0], NUM_UNIT],
                                               [1, BL * UNIT_SIZE]]))
                        break

                    # flatten v[n,(b,s)] -> vb[0, (b,n,s)], one DMA per n
                    pstV = v_t[:].ap[0][0]
                    pstVB = vb[:].ap[0][0]
                    for n in range(NUM_UNIT):
                        dstv = sub(vb[:], n * UNIT_SIZE,
                                   [[pstVB, 1], [NS, BL], [1, UNIT_SIZE]])
                        srcv = sub(v_t[:], n * pstV,
                                   [[pstV, 1], [UNIT_SIZE, BL],
                                    [1, UNIT_SIZE]])
                        nc.sync.dma_start(dstv, srcv)
                    nc.gpsimd.partition_broadcast(vb[:, :], vb[0:1, :])

                    # agreement: uv[c,n] = sum_{b,s} u_hat * v
                    QF = FREE // 4          # 1024 = 4 batches
                    for k in range(NCHUNK):
                        rsb = tb.tile([128, 4, 4, NUM_UNIT], f32, tag="rsb",
                                      name=f"rsb_{it}_{k}")
                        for h in range(4):
                            tmp = tb.tile([128, QF], f32, tag="uvt",
                                          name=f"uvt_{it}_{k}_{h}")
                            eng = (nc.gpsimd if (k * 4 + h) % 6 < 5
                                   else nc.vector)
                            eng.tensor_tensor(
                                tmp[:],
                                sub(Aap, k * FREE + h * QF,
                                    [[pstA, 128], [1, QF]], cast=f32),
                                vb[:, h * QF:(h + 1) * QF],
                                op=ALU.mult)
                            pstT = tmp[:].ap[0][0]
                            nc.vector.tensor_reduce(
                                rsb[:, h],
                                sub(tmp[:], 0,
                                    [[pstT, 128], [NS, 4],
                                     [UNIT_SIZE, NUM_UNIT], [1, UNIT_SIZE]]),
                                axis=AX.X, op=ALU.add)
                        pstR = rsb[:].ap[0][0]
                        nc.vector.tensor_reduce(
                            uv[:, k], sub(rsb[:], 0,
                                          [[pstR, 128], [1, NUM_UNIT],
                                           [NUM_UNIT, 16]]),
                            axis=AX.X, op=ALU.add)

                    arbounce_i = drampool.tile([128, NCHUNK * NUM_UNIT], f32,
                                               name=f"arbi_{it}", tag=f"arbi{it}")
                    arbounce_o = drampool.tile([128, NCHUNK * NUM_UNIT], f32,
                                               addr_space="Shared",
                                               name=f"arbo_{it}", tag=f"arbo{it}")
                    nc.gpsimd.dma_start(arbounce_i[:], uv[:])
                    if single_core:
                        nc.gpsimd.dma_start(arbounce_o[:], arbounce_i[:])
                    else:
                        nc.gpsimd.collective_compute(
                            "AllReduce", ALU.add,
                            replica_groups=[list(range(NCORES))],
                            ins=[arbounce_i.opt()], outs=[arbounce_o.opt()])
                    nc.sync.dma_start(ar_sb[:], arbounce_o[:])
                    # b_ij += AR/B
                    nc.vector.scalar_tensor_tensor(b_ij[:], ar_sb[:], 1.0 / B,
                                                   b_ij[:], op0=ALU.mult,
                                                   op1=ALU.add)

    nc.compile()
    return nc


def _prep(x, weight):
    wr = np.ascontiguousarray(
        weight.reshape(NGROUP, 8, NUM_UNIT, UNIT_SIZE, IN_UNIT)
        .transpose(0, 1, 4, 2, 3).reshape(NGROUP * 128, 256)).astype(np.float32)
    cij1 = np.full((128, NUM_UNIT), 1.0 / NUM_UNIT, np.float32)
    in_maps = []
    for c in range(NCORES):
        xs = x[c * BL:(c + 1) * BL]          # [BL, i, C]
        xc = np.ascontiguousarray(xs.transpose(2, 1, 0)).astype(np.float32)
        in_maps.append({"wr": wr, "xc": xc, "cij1": cij1})
    return in_maps


def kernel(x, x_original, weight, mode, epoch, _trace=False):
    from concourse.bass_utils import run_bass_kernel_spmd

    x = np.asarray(x, dtype=np.float32)
    weight = np.asarray(weight, dtype=np.float32)
    if "nc" not in _cache:
        _cache["nc"] = _build()
    nc = _cache["nc"]
    in_maps = _prep(x, weight)
    res = run_bass_kernel_spmd(nc, in_maps, core_ids=list(range(NCORES)),
                               trace=_trace)
    _cache["last_result"] = res
    out = np.empty((B, NUM_UNIT, UNIT_SIZE), np.float32)
    for c in range(NCORES):
        vo = res.results[c]["vout"].reshape(NUM_UNIT, BL, UNIT_SIZE)
        out[c * BL:(c + 1) * BL] = vo.transpose(1, 0, 2)
    return out[..., None]



# revision 21
# speedup vs baseline: 1.6785x; 1.6785x over previous
"""CapsuleLayer dynamic-routing kernel for 8 Trainium2 NeuronCores.

Sharding: data-parallel over batch (16 batches/core), weight replicated.
One AllReduce of the <u_hat,v> agreement per routing iteration (skipped on
the last).

v2 design (vs. DRAM-bounce baseline):
  - everything streamed in bf16 (weights, x, u_hat); fp32 accumulation in
    PSUM and for b_ij / squash.
  - u_hat produced per 128-channel chunk into PSUM [ (cc,b), (n,s) ],
    copied (cast) to SBUF, then re-partitioned to A[c%128, sg, b, n, s]
    with ONE SBUF->SBUF DMA per chunk (no DRAM bounce).
  - iter-0 s_j ((1/16)*sum_c u_hat) is accumulated on the PE during
    production with a block-diagonal ones stationary.
  - s_j iters 1,2: PE pass with c_ij stationary (c-partitioned A).
  - agreement <u_hat,v>: DVE/Pool elementwise multiply (bf16 2x mode),
    then sum over s on the PE via identity-stationary PSUM accumulation,
    then a tiny DVE reduce over b. uv lands directly in b_ij layout.
  - softmax exp and squash 1/sqrt (= exp(-0.5*ln)) both live in the
    natural_log_exp activation table set: one table load total.
"""

import sys

sys.path.insert(0, "/opt/trn_rl_repo")

import numpy as np

B, IN_UNIT, IN_CHANNEL = 128, 16, 1152
NUM_UNIT, UNIT_SIZE = 16, 16
NCORES = 8
BL = B // NCORES               # 16 batches per core
NCHUNK = IN_CHANNEL // 128     # 9 c-chunks
NS = NUM_UNIT * UNIT_SIZE      # 256
FREE = BL * NS                 # 4096 = (b, n, s) free size per chunk

_cache = {}


def _build(single_core=False, niters=3, dbg=False, no_cc=False):
    import concourse.bass as bass
    import concourse.bacc as bacc
    import concourse.mybir as mybir
    import concourse.tile as tile

    f32 = mybir.dt.float32
    bf = mybir.dt.bfloat16
    ALU = mybir.AluOpType
    AX = mybir.AxisListType
    ACT = mybir.ActivationFunctionType

    def sub(ap, off, dims, cast=None):
        a = bass.AP(ap.tensor, ap.offset + off, [list(d) for d in dims])
        return a.bitcast(cast) if cast is not None else a

    nc = bacc.Bacc("TRN2", target_bir_lowering=False, debug=False,
                   num_devices=1 if single_core else NCORES)

    wr_t = nc.dram_tensor("wr", [NCHUNK * 16 * 128, 256], bf, kind="ExternalInput")
    xc_t = nc.dram_tensor("xc", [NCHUNK, 128, 16 * 128], bf, kind="ExternalInput")
    ob_t = nc.dram_tensor("ob", [128, 16], bf, kind="ExternalInput")
    idn_t = nc.dram_tensor("idn", [128, 128], bf, kind="ExternalInput")
    vout_t = nc.dram_tensor("vout", [NUM_UNIT, BL * UNIT_SIZE], f32,
                            kind="ExternalOutput")
    if dbg:
        dbg_s0 = nc.dram_tensor("dbg_s0", [BL, NS], f32, kind="ExternalOutput")
        dbg_uv = nc.dram_tensor("dbg_uv", [128, NCHUNK * NUM_UNIT], f32,
                                kind="ExternalOutput")
        dbg_cij = nc.dram_tensor("dbg_cij", [128, NCHUNK * NUM_UNIT], f32,
                                 kind="ExternalOutput")
        dbg_A = nc.dram_tensor("dbg_A", [128, FREE], f32, kind="ExternalOutput")
        dbg_vb = nc.dram_tensor("dbg_vb", [128, FREE], f32, kind="ExternalOutput")

    with tile.TileContext(nc) as tc:
        with tc.tile_pool(name="apool", bufs=1) as apool, \
             tc.tile_pool(name="persist", bufs=1) as persist, \
             tc.tile_pool(name="drampool", bufs=1, space="DRAM") as drampool:
            # u_hat, bf16, c-partitioned: [c%128, chunk, b, n, s]
            A = apool.tile([128, NCHUNK, BL, NUM_UNIT, UNIT_SIZE], bf)
            Aap = A[:]
            pstA = Aap.ap[0][0]
            b_ij = persist.tile([128, NCHUNK, NUM_UNIT], f32)
            uv = persist.tile([128, NCHUNK, NUM_UNIT], f32)
            ob_sb = persist.tile([128, 16], bf)
            idn_sb = persist.tile([128, 128], bf)
            vb = persist.tile([128, BL, NUM_UNIT, UNIT_SIZE], bf)
            s0 = persist.tile([BL, NUM_UNIT, UNIT_SIZE], f32)  # iter-0 s_j
            nc.gpsimd.memset(b_ij[:], 0.0)
            nc.sync.dma_start(ob_sb[:], ob_t[:])
            nc.sync.dma_start(idn_sb[:], idn_t[:])
            vdram = drampool.tile([2, FREE], bf)   # v flatten bounce
            arb = []
            for it in range(niters - 1):
                arb.append((
                    drampool.tile([128, NCHUNK * NUM_UNIT], f32,
                                  name=f"arbi_{it}", tag=f"arbi{it}"),
                    drampool.tile([128, NCHUNK * NUM_UNIT], f32,
                                  addr_space="Shared",
                                  name=f"arbo_{it}", tag=f"arbo{it}")))

            # ---------------- production ----------------
            with tc.tile_pool(name="bdp", bufs=1) as bdp, \
                 tc.tile_pool(name="wp", bufs=1) as wp, \
                 tc.tile_pool(name="stgp", bufs=1) as stgp, \
                 tc.tile_pool(name="psp", bufs=4, space="PSUM") as psp, \
                 tc.tile_pool(name="ps0", bufs=1, space="PSUM") as ps0:
                sj0 = ps0.tile([BL, NUM_UNIT, UNIT_SIZE], f32)
                pst0 = sj0[:].ap[0][0]
                uhd = drampool.tile([NCHUNK * 128, FREE], bf)  # u_hat bounce
                for sg in range(NCHUNK):
                    # host-padded block-diag xT: bd16[(cc,i), g, cc*16+b]
                    bd16 = bdp.tile([128, 16, 128], bf, tag="bd",
                                    bufs=2, name=f"bd16_{sg}")
                    nc.sync.dma_start(bd16[:], xc_t[sg])
                    # weights for the chunk: one DMA -> [ (cc,i), g, (n,s) ]
                    wt = wp.tile([128, 16, 256], bf, tag="wt",
                                 bufs=2, name=f"wt_{sg}")
                    pstW = wt[:].ap[0][0]
                    nc.sync.dma_start(
                        wt[:], sub(wr_t[:], sg * 16 * 128 * 256,
                                   [[256, 128],              # (cc,i) rows
                                    [128 * 256, 16],         # g
                                    [1, 256]]))              # (n,s)
                    stg = stgp.tile([128, 16, 256], bf, tag="stg",
                                    bufs=2, name=f"stg_{sg}")
                    for gp in range(8):
                        ps = psp.tile([128, 2, 256], f32, tag="ps",
                                      name=f"ps_{sg}_{gp}")
                        for gi in range(2):
                            g = gp * 2 + gi
                            nc.tensor.matmul(ps[:, gi, :], bd16[:, g, :],
                                             wt[:, g, :],
                                             start=True, stop=True)
                        if gp % 2 == 0:
                            nc.vector.tensor_copy(
                                stg[:, gp * 2:gp * 2 + 2, :], ps[:])
                        else:
                            nc.scalar.copy(
                                stg[:, gp * 2:gp * 2 + 2, :], ps[:])
                        # iter-0 s_j: sj0[b,n,s] += (1/16)*sum_cc stg
                        for gi in range(2):
                            g = gp * 2 + gi
                            nc.tensor.matmul(sj0[:], ob_sb[:], stg[:, g, :],
                                             start=(sg == 0 and g == 0),
                                             stop=(sg == NCHUNK - 1 and g == 15))
                    # re-partition via DRAM (partition swaps can't be done
                    # SBUF->SBUF): uhd rows (g,cc,b) = chunk's partition-
                    # major mirror; readback rows map linearly to partition
                    # p' = g*8+cc of A.
                    pstS = stg[:].ap[0][0]
                    dstU = sub(uhd[:], sg * 128 * FREE,
                               [[256, 128],                  # (cc,b)
                                [128 * 256, 16],             # g
                                [1, 256]])                   # (n,s)
                    nc.sync.dma_start(dstU, sub(stg[:], 0,
                                                [[pstS, 128], [1, 4096]]))
                    dstA = sub(Aap, sg * FREE, [[pstA, 128], [1, FREE]])
                    srcU = sub(uhd[:], sg * 128 * FREE,
                               [[4096, 128], [1, 4096]])
                    nc.sync.dma_start(dstA, srcU)
                # sj0 psum -> sbuf
                nc.vector.tensor_copy(s0[:], sj0[:])
            if dbg:
                nc.sync.dma_start(dbg_s0[:],
                                  sub(s0[:], 0, [[s0[:].ap[0][0], BL], [1, NS]]))
                with tc.tile_pool(name="dbgp", bufs=1) as dbgp:
                    Af = dbgp.tile([128, FREE], f32)
                    nc.vector.tensor_copy(
                        Af[:], sub(Aap, 0, [[pstA, 128], [1, FREE]]))
                    nc.sync.dma_start(dbg_A[:], Af[:])

            # ---------------- routing ----------------
            with tc.tile_pool(name="rt", bufs=1) as rt, \
                 tc.tile_pool(name="tb", bufs=2) as tb, \
                 tc.tile_pool(name="prp", bufs=1) as prp, \
                 tc.tile_pool(name="pss", bufs=1, space="PSUM") as pss, \
                 tc.tile_pool(name="psr", bufs=2, space="PSUM") as psr:
                cij = rt.tile([128, NCHUNK, NUM_UNIT], f32)
                cij_bf = rt.tile([128, NCHUNK, NUM_UNIT], bf)
                smax = rt.tile([128, NCHUNK], f32)
                ssum = rt.tile([128, NCHUNK], f32)
                ar_sb = rt.tile([128, NCHUNK, NUM_UNIT], f32)

                for it in range(niters):
                    if it == 0:
                        # v from s0 [b, n, s] (squash over s)
                        red_view = s0[:]
                        part = BL
                    else:
                        # softmax over n of b_ij -> cij (bf16 copy for PE)
                        nc.vector.tensor_reduce(smax[:], b_ij[:], axis=AX.X,
                                                op=ALU.max)
                        mb = sub(smax[:], 0,
                                 [[NCHUNK, 128], [1, NCHUNK], [0, NUM_UNIT]])
                        nc.vector.tensor_tensor(cij[:], b_ij[:], mb,
                                                op=ALU.subtract)
                        nc.scalar.activation(cij[:], cij[:], ACT.Exp)
                        nc.vector.tensor_reduce(ssum[:], cij[:], axis=AX.X,
                                                op=ALU.add)
                        nc.vector.reciprocal(ssum[:], ssum[:])
                        sb = sub(ssum[:], 0,
                                 [[NCHUNK, 128], [1, NCHUNK], [0, NUM_UNIT]])
                        nc.vector.tensor_tensor(cij[:], cij[:], sb, op=ALU.mult)
                        nc.vector.tensor_copy(cij_bf[:], cij[:])
                        if dbg and it == 1:
                            nc.sync.dma_start(
                                dbg_cij[:],
                                sub(cij[:], 0, [[cij[:].ap[0][0], 128],
                                                [1, NCHUNK * NUM_UNIT]]))

                        # s_j: PE pass, two b-halves of PSUM [16, 2048]
                        sjf = tb.tile([NUM_UNIT, BL, UNIT_SIZE], f32,
                                      tag="sjf", name=f"sjf_{it}")
                        pstF = sjf[:].ap[0][0]
                        for h in range(2):
                            psj = pss.tile([NUM_UNIT, FREE // 2], f32,
                                           tag="psj", name=f"psj_{it}_{h}")
                            pstP = psj[:].ap[0][0]
                            for k in range(NCHUNK):
                                for j in range(4):
                                    mv = sub(Aap, k * FREE + h * 2048 + j * 512,
                                             [[pstA, 128], [1, 512]])
                                    nc.tensor.matmul(
                                        psj[:, j * 512:(j + 1) * 512],
                                        cij_bf[:, k, :], mv,
                                        start=(k == 0), stop=(k == NCHUNK - 1))
                            # psum -> sbuf (for the diag-extract DMA)
                            cp = tb.tile([NUM_UNIT, FREE // 2], f32,
                                         tag=f"cp{h}", name=f"cp_{it}_{h}")
                            if h == 0:
                                nc.vector.tensor_copy(cp[:], psj[:])
                            else:
                                nc.scalar.copy(cp[:], psj[:])
                            pstC = cp[:].ap[0][0]
                            # diagonal n'=n: sjf[n, h*8+b8, s] = cp[n,(b8,n,s)]
                            srcD = sub(cp[:], 0,
                                       [[pstC + UNIT_SIZE, NUM_UNIT],
                                        [NS, BL // 2], [1, UNIT_SIZE]])
                            dstD = sub(sjf[:], h * 8 * UNIT_SIZE,
                                       [[pstF, NUM_UNIT],
                                        [UNIT_SIZE, BL // 2],
                                        [0, 1],
                                        [1, UNIT_SIZE]])
                            nc.sync.dma_start(dstD, srcD)
                        red_view = sjf[:]      # [n, b, s]
                        part = NUM_UNIT

                    # squash: v = sq/(1+sq)/sqrt(sq) * s, over s (innermost)
                    s2 = tb.tile([part, 16, UNIT_SIZE], f32, tag="s2",
                                 name=f"s2_{it}")
                    nc.vector.tensor_tensor(s2[:], red_view, red_view,
                                            op=ALU.mult)
                    sq = tb.tile([part, 16], f32, tag="sq", name=f"sq_{it}")
                    nc.vector.tensor_reduce(sq[:], s2[:], axis=AX.X,
                                            op=ALU.add)
                    den = tb.tile([part, 16], f32, tag="den", name=f"den_{it}")
                    nc.vector.tensor_scalar_add(den[:], sq[:], 1.0)
                    nc.vector.reciprocal(den[:], den[:])
                    # 1/sqrt(sq): bit-hack seed + 2 Newton steps (DVE only;
                    # HW act tables lack ln, and Sqrt would thrash the Exp
                    # table set)
                    i32 = mybir.dt.int32
                    isq = tb.tile([part, 16], f32, tag="isq", name=f"isq_{it}")
                    nc.vector.tensor_scalar(isq[:].bitcast(i32),
                                            sq[:].bitcast(i32),
                                            1, 0xFFFFFFFF,
                                            op0=ALU.logical_shift_right,
                                            op1=ALU.bitwise_xor)
                    nc.vector.tensor_scalar_add(isq[:].bitcast(i32),
                                                isq[:].bitcast(i32),
                                                0x5F3759E0)
                    h = tb.tile([part, 16], f32, tag="h", name=f"h_{it}")
                    for _ in range(2):
                        nc.vector.tensor_tensor(h[:], isq[:], isq[:],
                                                op=ALU.mult)
                        nc.vector.tensor_tensor(h[:], h[:], sq[:], op=ALU.mult)
                        nc.vector.tensor_scalar(h[:], h[:], -0.5, 1.5,
                                                op0=ALU.mult, op1=ALU.add)
                        nc.vector.tensor_tensor(isq[:], isq[:], h[:],
                                                op=ALU.mult)
                    fac = tb.tile([part, 16], f32, tag="fac", name=f"fac_{it}")
                    nc.vector.tensor_tensor(fac[:], sq[:], den[:], op=ALU.mult)
                    nc.vector.tensor_tensor(fac[:], fac[:], isq[:], op=ALU.mult)
                    v_t = tb.tile([part, 16, UNIT_SIZE], f32, tag="v_t",
                                  name=f"v_t{it}")
                    pstV = v_t[:].ap[0][0]
                    fb = sub(fac[:], 0, [[fac[:].ap[0][0], part], [1, 16],
                                         [0, UNIT_SIZE]])
                    nc.vector.tensor_tensor(v_t[:], red_view, fb, op=ALU.mult)

                    if it == niters - 1:
                        # v_t is [n, (b,s)] here (niters>=2); write out
                        nc.sync.dma_start(
                            vout_t[:], sub(v_t[:], 0,
                                           [[pstV, NUM_UNIT], [1, 256]]))
                        break

                    # flatten to [1,(b,n,s)] bf16 + broadcast to 128 partitions
                    vbf = tb.tile([part, 16, UNIT_SIZE], bf, tag="vbf",
                                  name=f"vbf_{it}")
                    nc.vector.tensor_copy(vbf[:], v_t[:])
                    pstVB = vbf[:].ap[0][0]
                    pstVF = vb[:].ap[0][0]
                    vslot = it % 2
                    if it == 0:
                        # vbf [b, (n,s)] -> vdram[b*256+(n,s)]
                        srcV = sub(vbf[:], 0, [[pstVB, BL], [1, NS]])
                        dstV = sub(vdram[:], vslot * FREE, [[NS, BL], [1, NS]])
                    else:
                        # vbf [n, (b,s)] -> vdram[b*256+n*16+s]
                        srcV = sub(vbf[:], 0,
                                   [[pstVB, NUM_UNIT], [UNIT_SIZE, BL],
                                    [1, UNIT_SIZE]])
                        dstV = sub(vdram[:], vslot * FREE,
                                   [[UNIT_SIZE, NUM_UNIT], [NS, BL],
                                    [1, UNIT_SIZE]])
                    nc.scalar.dma_start(dstV, srcV)
                    # broadcast read: same DRAM row into all 128 partitions
                    nc.scalar.dma_start(
                        sub(vb[:], 0, [[pstVF, 128], [1, FREE]]),
                        sub(vdram[:], vslot * FREE, [[0, 128], [1, FREE]]))

                    # agreement: uv[c,(k,n)] = sum_{b,s} A * vb
                    for k in range(NCHUNK):
                        prod = prp.tile([128, BL, NUM_UNIT, UNIT_SIZE], bf,
                                        tag="prod", bufs=2,
                                        name=f"prod_{it}_{k}")
                        eng = nc.vector if k < 8 else nc.gpsimd
                        eng.tensor_tensor(prod[:], A[:, k], vb[:],
                                          op=ALU.mult)
                        pstQ = prod[:].ap[0][0]
                        r1 = psr.tile([128, 512], f32, tag="r1",
                                      name=f"r1_{it}_{k}")
                        for s2i in range(8):
                            # moving: (b, n, s-pair) -> [128, 512]
                            mv = sub(prod[:], s2i * 2,
                                     [[pstQ, 128], [256, BL],
                                      [16, NUM_UNIT], [1, 2]])
                            nc.tensor.matmul(r1[:], idn_sb[:], mv,
                                             start=(s2i == 0),
                                             stop=(s2i == 7))
                        # uv[:,k,n] = sum_{b,s2} r1[(b,n,s2)]
                        pstR = r1[:].ap[0][0]
                        rv = sub(r1[:], 0,
                                 [[pstR, 128], [2, NUM_UNIT],
                                  [32, BL], [1, 2]])
                        nc.vector.tensor_reduce(uv[:, k], rv, axis=AX.XY,
                                                op=ALU.add)

                    arbounce_i, arbounce_o = arb[it]
                    if dbg and it == 0:
                        nc.sync.dma_start(dbg_uv[:],
                                          sub(uv[:], 0, [[uv[:].ap[0][0], 128],
                                                         [1, NCHUNK * NUM_UNIT]]))
                        with tc.tile_pool(name="dbgv", bufs=1) as dbgv:
                            vbf32 = dbgv.tile([128, FREE], f32)
                            nc.vector.tensor_copy(
                                vbf32[:], sub(vb[:], 0, [[vb[:].ap[0][0], 128],
                                                         [1, FREE]]))
                            nc.sync.dma_start(dbg_vb[:], vbf32[:])
                    nc.gpsimd.dma_start(arbounce_i[:], uv[:])
                    if single_core or no_cc:
                        pass
                    else:
                        nc.gpsimd.collective_compute(
                            "AllReduce", ALU.add,
                            replica_groups=[list(range(NCORES))],
                            ins=[arbounce_i.opt()], outs=[arbounce_o.opt()])
                    nc.sync.dma_start(
                        ar_sb[:],
                        arbounce_i[:] if (single_core or no_cc)
                        else arbounce_o[:])
                    # b_ij += AR/B
                    nc.vector.scalar_tensor_tensor(
                        b_ij[:], ar_sb[:], 1.0 / B, b_ij[:],
                        op0=ALU.mult, op1=ALU.add)

    nc.compile()
    return nc


def _prep(x, weight):
    import concourse.mybir as mybir
    bfnp = mybir.dt.np(mybir.dt.bfloat16)
    # wr rows: (sg, g, cc, i) -> cols (n, s)
    wr = np.ascontiguousarray(
        weight.reshape(NCHUNK * 16, 8, NUM_UNIT, UNIT_SIZE, IN_UNIT)
        .transpose(0, 1, 4, 2, 3).reshape(NCHUNK * 16 * 128, 256)
    ).astype(bfnp)
    ob = np.zeros((128, 16), np.float32)
    for cc in range(8):
        for b in range(16):
            ob[cc * 16 + b, b] = 1.0 / 16.0
    ob = ob.astype(bfnp)
    idn = np.eye(128, dtype=np.float32).astype(bfnp)
    in_maps = []
    for c in range(NCORES):
        xs = x[c * BL:(c + 1) * BL]          # [BL, i, C]
        xcv = xs.transpose(2, 1, 0).reshape(NCHUNK, 16, 8, IN_UNIT, BL)
        # padded block-diag: xc[sg, (cc,i), g, cc*16 + b]
        xc = np.zeros((NCHUNK, 8, IN_UNIT, 16, 8, BL), np.float32)
        for cc in range(8):
            xc[:, cc, :, :, cc, :] = xcv[:, :, cc, :, :].transpose(0, 2, 1, 3)
        xc = np.ascontiguousarray(xc.reshape(NCHUNK, 128, 16 * 128)).astype(bfnp)
        in_maps.append({"wr": wr, "xc": xc, "ob": ob, "idn": idn})
    return in_maps


def kernel(x, x_original, weight, mode, epoch, _trace=False):
    from concourse.bass_utils import run_bass_kernel_spmd

    x = np.asarray(x, dtype=np.float32)
    weight = np.asarray(weight, dtype=np.float32)
    if "nc" not in _cache:
        _cache["nc"] = _build()
    nc = _cache["nc"]
    in_maps = _prep(x, weight)
    res = run_bass_kernel_spmd(nc, in_maps, core_ids=list(range(NCORES)),
                               trace=_trace)
    _cache["last_result"] = res
    out = np.empty((B, NUM_UNIT, UNIT_SIZE), np.float32)
    for c in range(NCORES):
        vo = res.results[c]["vout"].reshape(NUM_UNIT, BL, UNIT_SIZE)
        out[c * BL:(c + 1) * BL] = vo.transpose(1, 0, 2)
    return out[..., None]


# revision 31
# speedup vs baseline: 2.0877x; 1.2437x over previous
"""CapsuleLayer dynamic-routing kernel for 8 Trainium2 NeuronCores.

Sharding: data-parallel over batch (16 batches/core), weight replicated.
One AllReduce of the <u_hat,v> agreement per routing iteration (skipped on
the last).

v2 design (vs. DRAM-bounce baseline):
  - everything streamed in bf16 (weights, x, u_hat); fp32 accumulation in
    PSUM and for b_ij / squash.
  - u_hat produced per 128-channel chunk into PSUM [ (cc,b), (n,s) ],
    copied (cast) to SBUF, then re-partitioned to A[c%128, sg, b, n, s]
    with ONE SBUF->SBUF DMA per chunk (no DRAM bounce).
  - iter-0 s_j ((1/16)*sum_c u_hat) is accumulated on the PE during
    production with a block-diagonal ones stationary.
  - s_j iters 1,2: PE pass with c_ij stationary (c-partitioned A).
  - agreement <u_hat,v>: DVE/Pool elementwise multiply (bf16 2x mode),
    then sum over s on the PE via identity-stationary PSUM accumulation,
    then a tiny DVE reduce over b. uv lands directly in b_ij layout.
  - softmax exp and squash 1/sqrt (= exp(-0.5*ln)) both live in the
    natural_log_exp activation table set: one table load total.
"""

import sys

sys.path.insert(0, "/opt/trn_rl_repo")

import numpy as np

B, IN_UNIT, IN_CHANNEL = 128, 16, 1152
NUM_UNIT, UNIT_SIZE = 16, 16
NCORES = 8
BL = B // NCORES               # 16 batches per core
NCHUNK = IN_CHANNEL // 128     # 9 c-chunks
NS = NUM_UNIT * UNIT_SIZE      # 256
FREE = BL * NS                 # 4096 = (b, n, s) free size per chunk

_cache = {}


def _build(single_core=False, niters=3, dbg=False, no_cc=False):
    import concourse.bass as bass
    import concourse.bacc as bacc
    import concourse.mybir as mybir
    import concourse.tile as tile

    f32 = mybir.dt.float32
    bf = mybir.dt.bfloat16
    ALU = mybir.AluOpType
    AX = mybir.AxisListType
    ACT = mybir.ActivationFunctionType

    def sub(ap, off, dims, cast=None):
        a = bass.AP(ap.tensor, ap.offset + off, [list(d) for d in dims])
        return a.bitcast(cast) if cast is not None else a

    nc = bacc.Bacc("TRN2", target_bir_lowering=False, debug=False,
                   num_devices=1 if single_core else NCORES)

    wr_t = nc.dram_tensor("wr", [NCHUNK * 16 * 128, 256], bf, kind="ExternalInput")
    xc_t = nc.dram_tensor("xc", [NCHUNK, 128, 16 * 128], bf, kind="ExternalInput")
    ob_t = nc.dram_tensor("ob", [128, 16], bf, kind="ExternalInput")
    idn_t = nc.dram_tensor("idn", [128, 128], bf, kind="ExternalInput")
    vout_t = nc.dram_tensor("vout", [NUM_UNIT, BL * UNIT_SIZE], f32,
                            kind="ExternalOutput")
    if dbg:
        dbg_s0 = nc.dram_tensor("dbg_s0", [BL, NS], f32, kind="ExternalOutput")
        dbg_uv = nc.dram_tensor("dbg_uv", [128, NCHUNK * NUM_UNIT], f32,
                                kind="ExternalOutput")
        dbg_cij = nc.dram_tensor("dbg_cij", [128, NCHUNK * NUM_UNIT], f32,
                                 kind="ExternalOutput")
        dbg_A = nc.dram_tensor("dbg_A", [128, FREE], f32, kind="ExternalOutput")
        dbg_vb = nc.dram_tensor("dbg_vb", [128, FREE], f32, kind="ExternalOutput")

    with tile.TileContext(nc) as tc:
        with tc.tile_pool(name="apool", bufs=1) as apool, \
             tc.tile_pool(name="persist", bufs=1) as persist, \
             tc.tile_pool(name="drampool", bufs=1, space="DRAM") as drampool:
            # u_hat, bf16, c-partitioned: [c%128, chunk, b, n, s]
            A = apool.tile([128, NCHUNK, BL, NUM_UNIT, UNIT_SIZE], bf)
            Aap = A[:]
            pstA = Aap.ap[0][0]
            b_ij = persist.tile([128, NCHUNK, NUM_UNIT], f32)
            uv = persist.tile([128, NCHUNK, NUM_UNIT], f32)
            ob_sb = persist.tile([128, 16], bf)
            idn_sb = persist.tile([128, 128], bf)
            vb = persist.tile([128, BL, NUM_UNIT, UNIT_SIZE], bf)
            s0 = persist.tile([BL, NUM_UNIT, UNIT_SIZE], f32)  # iter-0 s_j
            nc.gpsimd.memset(b_ij[:], 0.0)
            nc.sync.dma_start(ob_sb[:], ob_t[:])
            nc.sync.dma_start(idn_sb[:], idn_t[:])
            vdram = drampool.tile([2, FREE], bf)   # v flatten bounce
            arb = []
            for it in range(niters - 1):
                arb.append((
                    drampool.tile([128, NCHUNK * NUM_UNIT], f32,
                                  name=f"arbi_{it}", tag=f"arbi{it}"),
                    drampool.tile([128, NCHUNK * NUM_UNIT], f32,
                                  addr_space="Shared",
                                  name=f"arbo_{it}", tag=f"arbo{it}")))

            # ---------------- production ----------------
            with tc.tile_pool(name="bdp", bufs=1) as bdp, \
                 tc.tile_pool(name="wp", bufs=1) as wp, \
                 tc.tile_pool(name="stgp", bufs=1) as stgp, \
                 tc.tile_pool(name="psp", bufs=4, space="PSUM") as psp, \
                 tc.tile_pool(name="ps0", bufs=1, space="PSUM") as ps0:
                sj0 = ps0.tile([BL, 2, NS], f32)
                uhd = drampool.tile([NCHUNK * 128, FREE], bf)  # u_hat bounce
                for sg in range(NCHUNK):
                    # host-padded block-diag xT: bd16[(cc,i), g, cc*16+b]
                    bd16 = bdp.tile([128, 16, 128], bf, tag="bd",
                                    bufs=3, name=f"bd16_{sg}")
                    nc.sync.dma_start(bd16[:], xc_t[sg])
                    # weights for the chunk: one DMA -> [ (cc,i), g, (n,s) ]
                    wt = wp.tile([128, 16, 256], bf, tag="wt",
                                 bufs=3, name=f"wt_{sg}")
                    pstW = wt[:].ap[0][0]
                    nc.sync.dma_start(
                        wt[:], sub(wr_t[:], sg * 16 * 128 * 256,
                                   [[256, 128],              # (cc,i) rows
                                    [128 * 256, 16],         # g
                                    [1, 256]]))              # (n,s)
                    stg = stgp.tile([128, 16, 256], bf, tag="stg",
                                    bufs=2, name=f"stg_{sg}")
                    for gp in range(8):
                        ps = psp.tile([128, 2, 256], f32, tag="ps",
                                      name=f"ps_{sg}_{gp}")
                        for gi in range(2):
                            g = gp * 2 + gi
                            nc.tensor.matmul(ps[:, gi, :], bd16[:, g, :],
                                             wt[:, g, :],
                                             start=True, stop=True)
                        if gp % 2 == 0:
                            nc.vector.tensor_copy(
                                stg[:, gp * 2:gp * 2 + 2, :], ps[:])
                        else:
                            nc.scalar.copy(
                                stg[:, gp * 2:gp * 2 + 2, :], ps[:])
                        # iter-0 s_j: sj0[b,par,n,s] += (1/16)*sum_cc stg
                        nc.tensor.matmul(sj0[:], ob_sb[:],
                                         stg[:, gp * 2:gp * 2 + 2, :],
                                         start=(sg == 0 and gp == 0),
                                         stop=(sg == NCHUNK - 1 and gp == 7))
                    # re-partition via DRAM (partition swaps can't be done
                    # SBUF->SBUF): uhd rows (g,cc,b) = chunk's partition-
                    # major mirror; readback rows map linearly to partition
                    # p' = g*8+cc of A.
                    pstS = stg[:].ap[0][0]
                    dstU = sub(uhd[:], sg * 128 * FREE,
                               [[256, 128],                  # (cc,b)
                                [128 * 256, 16],             # g
                                [1, 256]])                   # (n,s)
                    nc.sync.dma_start(dstU, sub(stg[:], 0,
                                                [[pstS, 128], [1, 4096]]))
                    dstA = sub(Aap, sg * FREE, [[pstA, 128], [1, FREE]])
                    srcU = sub(uhd[:], sg * 128 * FREE,
                               [[4096, 128], [1, 4096]])
                    nc.sync.dma_start(dstA, srcU)
                # sj0 psum -> sbuf (sum the two parity slots)
                pstJ0 = sj0[:].ap[0][0]
                nc.vector.tensor_copy(
                    s0[:], sub(sj0[:], 0, [[pstJ0, BL], [16, NUM_UNIT],
                                           [1, UNIT_SIZE]]))
                nc.vector.tensor_tensor(
                    s0[:], s0[:],
                    sub(sj0[:], NS, [[pstJ0, BL], [16, NUM_UNIT],
                                     [1, UNIT_SIZE]]),
                    op=ALU.add)
            if dbg:
                nc.sync.dma_start(dbg_s0[:],
                                  sub(s0[:], 0, [[s0[:].ap[0][0], BL], [1, NS]]))
                with tc.tile_pool(name="dbgp", bufs=1) as dbgp:
                    Af = dbgp.tile([128, FREE], f32)
                    nc.vector.tensor_copy(
                        Af[:], sub(Aap, 0, [[pstA, 128], [1, FREE]]))
                    nc.sync.dma_start(dbg_A[:], Af[:])

            # ---------------- routing ----------------
            with tc.tile_pool(name="rt", bufs=1) as rt, \
                 tc.tile_pool(name="tb", bufs=2) as tb, \
                 tc.tile_pool(name="prp", bufs=1) as prp, \
                 tc.tile_pool(name="pss", bufs=1, space="PSUM") as pss, \
                 tc.tile_pool(name="psr", bufs=2, space="PSUM") as psr:
                wn = [0]

                def warm(dep1, dep16):
                    # tiny PE matmul chained to a late tile: keeps the PE
                    # p-state ramp warm across engine-idle windows
                    wn[0] += 1
                    psw = psr.tile([1, 16], f32, tag="warm",
                                   name=f"warm_{wn[0]}")
                    nc.tensor.matmul(psw[:, 0:dep16.free_size()], dep1, dep16,
                                     start=True, stop=True)

                cij = rt.tile([128, NCHUNK, NUM_UNIT], f32)
                cij_bf = rt.tile([128, NCHUNK, NUM_UNIT], bf)
                smax = rt.tile([128, NCHUNK], f32)
                ssum = rt.tile([128, NCHUNK], f32)
                ar_sb = rt.tile([128, NCHUNK, NUM_UNIT], f32)

                for it in range(niters):
                    if it > 0:
                        # softmax over n of b_ij -> cij (bf16 copy for PE)
                        nc.scalar.activation(cij[:], b_ij[:], ACT.Exp)
                        nc.vector.tensor_reduce(ssum[:], cij[:], axis=AX.X,
                                                op=ALU.add)
                        nc.vector.reciprocal(ssum[:], ssum[:])
                        sb = sub(ssum[:], 0,
                                 [[NCHUNK, 128], [1, NCHUNK], [0, NUM_UNIT]])
                        nc.vector.tensor_tensor(cij[:], cij[:], sb, op=ALU.mult)
                        nc.vector.tensor_copy(cij_bf[:], cij[:])
                        warm(cij[:, 0, 0:1], cij[:, 0, :])
                        if dbg and it == 1:
                            nc.sync.dma_start(
                                dbg_cij[:],
                                sub(cij[:], 0, [[cij[:].ap[0][0], 128],
                                                [1, NCHUNK * NUM_UNIT]]))
                        sjf = tb.tile([NUM_UNIT, BL, UNIT_SIZE], f32,
                                      tag="sjf", name=f"sjf_{it}")
                        pstF = sjf[:].ap[0][0]

                    last = it == niters - 1
                    vslot = it % 2
                    pstVF = vb[:].ap[0][0]
                    i32 = mybir.dt.int32
                    if it > 0:
                        # s_j in 4 b-quarters of PSUM [16, 1024]
                        for q in range(4):
                            psj = pss.tile([NUM_UNIT, FREE // 4], f32,
                                           tag=f"psjq{q % 2}",
                                           name=f"psj_{it}_{q}")
                            for k in range(NCHUNK):
                                for j in range(2):
                                    mv = sub(Aap,
                                             k * FREE + q * 1024 + j * 512,
                                             [[pstA, 128], [1, 512]])
                                    nc.tensor.matmul(
                                        psj[:, j * 512:(j + 1) * 512],
                                        cij_bf[:, k, :], mv,
                                        start=(k == 0), stop=(k == NCHUNK - 1))
                            # psum -> sbuf (for the diag-extract DMA)
                            cp = tb.tile([NUM_UNIT, FREE // 4], f32,
                                         tag=f"cpq{q % 2}", name=f"cp_{it}_{q}")
                            if q % 2 == 0:
                                nc.vector.tensor_copy(cp[:], psj[:])
                            else:
                                nc.scalar.copy(cp[:], psj[:])
                            if q == 3:
                                warm(cp[:, 0:1], cp[:, 0:16])
                            pstC = cp[:].ap[0][0]
                            # diag n'=n: sjf[n, q*4+b4, s] = cp[n,(b4,n,s)]
                            srcD = sub(cp[:], 0,
                                       [[pstC + UNIT_SIZE, NUM_UNIT],
                                        [NS, BL // 4], [1, UNIT_SIZE]])
                            dstD = sub(sjf[:], q * 4 * UNIT_SIZE,
                                       [[pstF, NUM_UNIT],
                                        [UNIT_SIZE, BL // 4],
                                        [1, UNIT_SIZE]])
                            nc.sync.dma_start(dstD, srcD)
                    for hh in ((0, 1) if it > 0 else (2,)):
                        if it > 0:
                            red_h = sjf[:, hh * 8:(hh + 1) * 8, :]
                            part = NUM_UNIT
                            w16 = 8
                        else:
                            # full-width pass (engine slices must start at
                            # partition 0, so s0 can't be split)
                            red_h = s0[:]
                            part = BL
                            w16 = 16

                        # squash half: v = sq/(1+sq)/sqrt(sq) * s  (s inner)
                        s2 = tb.tile([part, w16, UNIT_SIZE], f32,
                                     tag=f"s2{hh}", name=f"s2_{it}_{hh}")
                        nc.vector.tensor_tensor(s2[:], red_h, red_h,
                                                op=ALU.mult)
                        sq = tb.tile([part, w16], f32, tag=f"sq{hh}",
                                     name=f"sq_{it}_{hh}")
                        nc.vector.tensor_reduce(sq[:], s2[:], axis=AX.X,
                                                op=ALU.add)
                        if hh == 1:
                            warm(sq[:, 0:1], sq[:])
                        den = tb.tile([part, w16], f32, tag=f"den{hh}",
                                      name=f"den_{it}_{hh}")
                        nc.vector.tensor_scalar_add(den[:], sq[:], 1.0)
                        nc.vector.reciprocal(den[:], den[:])
                        # rsqrt: bit-hack seed + 2 Newton steps (DVE only; HW
                        # act tables lack ln, Sqrt would thrash the Exp set)
                        isq = tb.tile([part, w16], f32, tag=f"isq{hh}",
                                      name=f"isq_{it}_{hh}")
                        nc.vector.tensor_scalar(isq[:].bitcast(i32),
                                                sq[:].bitcast(i32),
                                                1, 0xFFFFFFFF,
                                                op0=ALU.logical_shift_right,
                                                op1=ALU.bitwise_xor)
                        nc.vector.tensor_scalar_add(isq[:].bitcast(i32),
                                                    isq[:].bitcast(i32),
                                                    0x5F3759E0)
                        nh = tb.tile([part, w16], f32, tag=f"nh{hh}",
                                     name=f"nh_{it}_{hh}")
                        for _ in range(2):
                            nc.vector.tensor_tensor(nh[:], isq[:], isq[:],
                                                    op=ALU.mult)
                            nc.vector.tensor_tensor(nh[:], nh[:], sq[:],
                                                    op=ALU.mult)
                            nc.vector.tensor_scalar(nh[:], nh[:], -0.5, 1.5,
                                                    op0=ALU.mult, op1=ALU.add)
                            nc.vector.tensor_tensor(isq[:], isq[:], nh[:],
                                                    op=ALU.mult)
                        fac = tb.tile([part, w16], f32, tag=f"fac{hh}",
                                      name=f"fac_{it}_{hh}")
                        nc.vector.tensor_tensor(fac[:], sq[:], den[:],
                                                op=ALU.mult)
                        nc.vector.tensor_tensor(fac[:], fac[:], isq[:],
                                                op=ALU.mult)
                        if hh == 1:
                            warm(fac[:, 0:1], fac[:])
                        v_t = tb.tile([part, w16, UNIT_SIZE], f32,
                                      tag=f"v_t{hh}", name=f"v_t{it}_{hh}")
                        pstV = v_t[:].ap[0][0]
                        fb = sub(fac[:], 0, [[fac[:].ap[0][0], part], [1, w16],
                                             [0, UNIT_SIZE]])
                        nc.vector.tensor_tensor(v_t[:], red_h, fb, op=ALU.mult)

                        if last:
                            # v_t is [n, b-half, s]; write out the half
                            nc.sync.dma_start(
                                sub(vout_t[:], hh * 8 * UNIT_SIZE,
                                    [[256, NUM_UNIT], [1, 128]]),
                                sub(v_t[:], 0, [[pstV, NUM_UNIT], [1, 128]]))
                            continue
                        if hh == 2:
                            # iter 0: flatten full, broadcast in two halves
                            vbf = tb.tile([part, w16, UNIT_SIZE], bf,
                                          tag="vbf2", name=f"vbf_{it}_f")
                            nc.vector.tensor_copy(vbf[:], v_t[:])
                            warm(vbf[:, 0, 0:1], vbf[:, 0, :])
                            pstVB = vbf[:].ap[0][0]
                            srcV = sub(vbf[:], 0, [[pstVB, BL], [1, NS]])
                            dstV = sub(vdram[:], vslot * FREE,
                                       [[NS, BL], [1, NS]])
                            nc.sync.dma_start(dstV, srcV)
                            for h2 in range(2):
                                nc.sync.dma_start(
                                    sub(vb[:], h2 * 2048,
                                        [[pstVF, 128], [1, 2048]]),
                                    sub(vdram[:], vslot * FREE + h2 * 2048,
                                        [[0, 128], [1, 2048]]))
                            warm(vb[:, 0, 0, 0:1], vb[:, 0, 0, :])
                            continue

                        # flatten half to DRAM (b,n,s) + broadcast to vb
                        vbf = tb.tile([part, w16, UNIT_SIZE], bf,
                                      tag=f"vbf{hh}", name=f"vbf_{it}_{hh}")
                        nc.vector.tensor_copy(vbf[:], v_t[:])
                        if hh == 1:
                            warm(vbf[:, 0, 0:1], vbf[:, 0, :])
                        pstVB = vbf[:].ap[0][0]
                        # vbf [n, b-h, s] -> vdram[b*256+n*16+s]
                        srcV = sub(vbf[:], 0,
                                   [[pstVB, NUM_UNIT], [UNIT_SIZE, 8],
                                    [1, UNIT_SIZE]])
                        dstV = sub(vdram[:], vslot * FREE + hh * 2048,
                                   [[UNIT_SIZE, NUM_UNIT], [NS, 8],
                                    [1, UNIT_SIZE]])
                        nc.sync.dma_start(dstV, srcV)
                        nc.sync.dma_start(
                            sub(vb[:], hh * 2048, [[pstVF, 128], [1, 2048]]),
                            sub(vdram[:], vslot * FREE + hh * 2048,
                                [[0, 128], [1, 2048]]))
                        if hh == 1:
                            warm(vb[:, 0, 0, 0:1], vb[:, 0, 0, :])

                    if last:
                        break

                    # agreement: uv[c,(k,n)] = sum_{b,s} A * vb
                    for k in range(NCHUNK):
                        prod = prp.tile([128, BL, NUM_UNIT, UNIT_SIZE], bf,
                                        tag="prod", bufs=3,
                                        name=f"prod_{it}_{k}")
                        eng = nc.vector if k < 8 else nc.gpsimd
                        for hh in range(2):
                            eng.tensor_tensor(
                                prod[:, hh * 8:(hh + 1) * 8],
                                A[:, k, hh * 8:(hh + 1) * 8],
                                vb[:, hh * 8:(hh + 1) * 8], op=ALU.mult)
                        pstQ = prod[:].ap[0][0]
                        warm(prod[:, 0, 0, 0:1], prod[:, 0, 0, :])
                        if True:
                            # PE: sum over s via identity-psum accumulation
                            r1 = psr.tile([128, 512], f32, tag="r1",
                                          name=f"r1_{it}_{k}")
                            for s2i in range(8):
                                mv = sub(prod[:], s2i * 2,
                                         [[pstQ, 128], [256, BL],
                                          [16, NUM_UNIT], [1, 2]])
                                nc.tensor.matmul(r1[:], idn_sb[:], mv,
                                                 start=(s2i == 0),
                                                 stop=(s2i == 7))
                            pstR = r1[:].ap[0][0]
                            rv = sub(r1[:], 0,
                                     [[pstR, 128], [2, NUM_UNIT],
                                      [32, BL], [1, 2]])
                            nc.vector.tensor_reduce(uv[:, k], rv, axis=AX.XY,
                                                    op=ALU.add)
                        else:
                            # DVE: single fused (b,s)-reduce from prod
                            rv = sub(prod[:], 0,
                                     [[pstQ, 128], [16, NUM_UNIT],
                                      [256, BL], [1, UNIT_SIZE]])
                            nc.vector.tensor_reduce(uv[:, k], rv, axis=AX.XY,
                                                    op=ALU.add)

                    arbounce_i, arbounce_o = arb[it]
                    if dbg and it == 0:
                        nc.sync.dma_start(dbg_uv[:],
                                          sub(uv[:], 0, [[uv[:].ap[0][0], 128],
                                                         [1, NCHUNK * NUM_UNIT]]))
                        with tc.tile_pool(name="dbgv", bufs=1) as dbgv:
                            vbf32 = dbgv.tile([128, FREE], f32)
                            nc.vector.tensor_copy(
                                vbf32[:], sub(vb[:], 0, [[vb[:].ap[0][0], 128],
                                                         [1, FREE]]))
                            nc.sync.dma_start(dbg_vb[:], vbf32[:])
                    nc.gpsimd.dma_start(arbounce_i[:], uv[:])
                    if single_core or no_cc:
                        pass
                    else:
                        nc.gpsimd.collective_compute(
                            "AllReduce", ALU.add,
                            replica_groups=[list(range(NCORES))],
                            ins=[arbounce_i.opt()], outs=[arbounce_o.opt()])
                    nc.sync.dma_start(
                        ar_sb[:],
                        arbounce_i[:] if (single_core or no_cc)
                        else arbounce_o[:])
                    warm(ar_sb[:, 0, 0:1], ar_sb[:, 0, :])
                    # b_ij += AR/B
                    nc.vector.scalar_tensor_tensor(
                        b_ij[:], ar_sb[:], 1.0 / B, b_ij[:],
                        op0=ALU.mult, op1=ALU.add)
                    warm(b_ij[:, 0, 0:1], b_ij[:, 0, :])

    nc.compile()
    return nc


def _prep(x, weight):
    import concourse.mybir as mybir
    bfnp = mybir.dt.np(mybir.dt.bfloat16)
    # wr rows: (sg, g, cc, i) -> cols (n, s)
    wr = np.ascontiguousarray(
        weight.reshape(NCHUNK * 16, 8, NUM_UNIT, UNIT_SIZE, IN_UNIT)
        .transpose(0, 1, 4, 2, 3).reshape(NCHUNK * 16 * 128, 256)
    ).astype(bfnp)
    ob = np.zeros((128, 16), np.float32)
    for cc in range(8):
        for b in range(16):
            ob[cc * 16 + b, b] = 1.0 / 16.0
    ob = ob.astype(bfnp)
    idn = np.eye(128, dtype=np.float32).astype(bfnp)
    in_maps = []
    for c in range(NCORES):
        xs = x[c * BL:(c + 1) * BL]          # [BL, i, C]
        xcv = xs.transpose(2, 1, 0).reshape(NCHUNK, 16, 8, IN_UNIT, BL)
        # padded block-diag: xc[sg, (cc,i), g, cc*16 + b]
        xc = np.zeros((NCHUNK, 8, IN_UNIT, 16, 8, BL), np.float32)
        for cc in range(8):
            xc[:, cc, :, :, cc, :] = xcv[:, :, cc, :, :].transpose(0, 2, 1, 3)
        xc = np.ascontiguousarray(xc.reshape(NCHUNK, 128, 16 * 128)).astype(bfnp)
        in_maps.append({"wr": wr, "xc": xc, "ob": ob, "idn": idn})
    return in_maps


def kernel(x, x_original, weight, mode, epoch, _trace=False):
    from concourse.bass_utils import run_bass_kernel_spmd

    x = np.asarray(x, dtype=np.float32)
    weight = np.asarray(weight, dtype=np.float32)
    if "nc" not in _cache:
        _cache["nc"] = _build()
    nc = _cache["nc"]
    in_maps = _prep(x, weight)
    res = run_bass_kernel_spmd(nc, in_maps, core_ids=list(range(NCORES)),
                               trace=_trace)
    _cache["last_result"] = res
    out = np.empty((B, NUM_UNIT, UNIT_SIZE), np.float32)
    for c in range(NCORES):
        vo = res.results[c]["vout"].reshape(NUM_UNIT, BL, UNIT_SIZE)
        out[c * BL:(c + 1) * BL] = vo.transpose(1, 0, 2)
    return out[..., None]
